# revision 6
# baseline (speedup 1.0000x reference)
"""Trainium2 Bass kernel for CGL contrastive region loss — v3.

Problem: proj (96, 256, 64, 64) f32 = 3 stacked views of B=32 images.
Views 2/3 are used; 25 regions (5x5 grid of 2x2 windows, all 256 chans)
per image -> region vectors D=1024. Per pair the loss needs the 50x50
Gram of [u1;u2] normalized rows, two masked logsumexps, positives.
Scalar loss sums over pairs -> data-parallel over batch, 4 pairs/core.

vs the 35us v1:
  - inputs packed bf16 on host: halves DMA bytes; bf16 gram matmuls
    (1-pass, ~2.5x the fp32 HIGH rate).
  - input is two half-transfers, one per HWDGE ring (a second DMA in
    the same ring lands 10.3-12.4us, pure lottery; one big transfer
    per ring lands ~10.2us reliably with double-size descriptors).
    Const traffic is one 10KB (50,50) identity: 4-block views are
    stride-0 broadcast APs, the ones matrix is a memset.
  - all 4 pair grams accumulate into one (50,200) PSUM tile; the diag
    extraction / row-scale / mask / exp run as single 200-wide ops.
  - rsqrt via ACT: inv = exp(-0.5*ln(0.1*d)) — two tiny activations
    replacing the 12-op DVE Newton chain. The logsumexp diag-kill is a
    -1e38 poison written onto the gram diagonals DURING those
    activations (logits are bounded by 10 so no max-shift is needed,
    and ACT-table inv error ~1e-3 rules out analytic diag
    cancellation — it must be hard-killed).
  - insert_act_table_loads is steered to the one act-func set that
    contains BOTH exp and ln ("natural_log_exp_and_others"), so the
    kernel does exactly one table load, hoisted behind a no-wait dummy
    exp into the DMA window (v1/v2 paid 1.3-2.6us of mid-chain loads).
  - eall is bf16 (2x DVE reduce rate); the positives leave early on
    the idle gpsimd DMA queue so the two output HBM receipts overlap;
    per-core output is the (50,8) [lnes | posf] tile and final scalar
    assembly happens on the host during the gather step.
"""

import numpy as np
import ml_dtypes

NB = 4                    # pairs per core
NCORES = 8
R = 25
FREE = NB * 8 * 50        # 1600 free elements per core
Q = FREE // NB            # 400 per pair
_CENTRES = (10, 20, 30, 40, 50)
_BOTH_SET = "natural_log_exp_and_others"

_nc_cache = None


def _build_nc():
    import concourse.bacc as bacc
    import concourse.tile as tile
    from concourse import mybir
    from concourse.hw_specs import get_activation_tables
    from concourse.vector_clock import ScopedClock

    class FastTailTileContext(tile.TileContext):
        """Tile tail without the two full all-engine barriers.

        The sync-engine drain already waits on the global vector clock
        (every instruction's sem tick), so once it completes nothing is
        in flight; a sem-only EVSEM barrier then orders the gpsimd
        sem_clears after it. Saves ~6us of kernel tail."""

        def _drain_and_barrier(self, tick_clock, wait_clock):
            drain_inst = self.nc.sync.drain()
            wait_clock.add_sem_waits(
                drain_inst.ins, ScopedClock({None: tick_clock.global_clock})
            )
            self.nc.all_engine_barrier(sem_only=True)
            popped = self.nc._tile_sem_poison_stack.pop()
            assert popped is self._sem_poison
            self.nc.clear_and_free_semaphores(list(self.sems.allocated().values()))

    class OneActSetBacc(bacc.Bacc):
        """Steer activation-table selection to the single set holding
        both exp and ln, so the kernel needs exactly one table load.

        The act_func_set_id written on InstLoadActFuncSet is the INDEX
        into act_info.json's act_func_sets, so the list order must be
        preserved — other sets are emptied, not removed, which makes
        them unselectable without disturbing the indices."""

        def insert_act_table_loads(self):
            has_activation = any(
                isinstance(i, mybir.InstActivation)
                for b in self.main_func.blocks
                for i in b.instructions
            )
            if not has_activation:
                return
            tables = [
                (name, funcs if name == _BOTH_SET else set())
                for name, funcs in get_activation_tables(self.m.arch).items()
            ]
            bacc._bass_rust.insert_act_table_loads(self, tables)

    f32 = mybir.dt.float32
    bf16 = mybir.dt.bfloat16
    Act = mybir.ActivationFunctionType
    Alu = mybir.AluOpType
    X = mybir.AxisListType.X

    nc = OneActSetBacc("TRN2", target_bir_lowering=False, debug=False)
    u_dram = nc.dram_tensor("u", [128, FREE], bf16, kind="ExternalInput").ap()
    cf_dram = nc.dram_tensor("cf", [50, 50], f32, kind="ExternalInput").ap()
    out_dram = nc.dram_tensor("out", [50, 8], f32, kind="ExternalOutput").ap()

    def blk(ap, f=50):
        return ap.rearrange("p (b f) -> p b f", f=f)

    with FastTailTileContext(nc) as tc:
        with (
            tc.tile_pool(name="data", bufs=1) as data,
            tc.tile_pool(name="consts", bufs=1) as consts,
            tc.tile_pool(name="work", bufs=2) as work,
            tc.tile_pool(name="psg", bufs=1, space="PSUM") as psg,
            tc.tile_pool(name="psb", bufs=1, space="PSUM") as psb,
        ):
            # input DMAs: one half per HWDGE ring (pairs 0+1 on sync,
            # 2+3 on scalar). A single transfer per ring avoids the
            # highly variable landing time of a second-in-queue DMA
            # (observed 10.3-12.4us) and doubles the descriptor size
            uh = [
                data.tile([128, 2 * Q], bf16, name=f"uh{h}", tag=f"uh{h}")
                for h in range(2)
            ]
            nc.sync.dma_start(uh[0][:], u_dram[:, 0 : 2 * Q])
            nc.scalar.dma_start(uh[1][:], u_dram[:, 2 * Q : 4 * Q])
            ubs = [uh[b // 2][:, (b % 2) * Q : (b % 2 + 1) * Q] for b in range(NB)]

            # dummy exp on a constant tile: hoists the single ACT table
            # load into the DMA window instead of the post-gram chain
            dummy = consts.tile([1, 1], f32)
            nc.vector.memset(dummy[:], 1.0)
            dume = work.tile([1, 1], f32, tag="dume")
            nc.scalar.activation(dume[:], dummy[:], Act.Exp)

            # consts: (50,50) identity from DRAM; 4-block views are
            # stride-0 broadcast APs
            ident = consts.tile([50, 50], f32)
            nc.gpsimd.dma_start(ident[:], cf_dram)
            identB = ident[:].unsqueeze(1).broadcast_to([50, NB, 50])
            cb = consts.tile([50, 50], bf16)
            nc.vector.memset(cb[:], 1.0)

            # output tile; posf lands in cols 4:8 (rows 25:50 stay 0)
            cmb = consts.tile([50, 8], f32)
            nc.vector.memset(cmb[:, 4:8], 0.0)

            # grams: 4 accumulation groups into one (50,200) PSUM tile
            gp = psg.tile([50, 200], f32, tag="g")
            for b in range(NB):
                for k in range(8):
                    sl = ubs[b][:, k * 50 : (k + 1) * 50]
                    nc.tensor.matmul(
                        gp[:, b * 50 : (b + 1) * 50], sl, sl,
                        start=(k == 0), stop=(k == 7),
                    )

            # squared norms from the block diagonals
            dmul = work.tile([50, 200], f32, tag="dmul")
            nc.vector.tensor_mul(blk(dmul[:]), blk(gp[:]), identB)
            dsq = work.tile([50, NB], f32, tag="dsq")
            nc.vector.reduce_sum(dsq[:], blk(dmul[:]), axis=X)

            # inv = sqrt(10)*rsqrt(d) = exp(-0.5*ln(0.1*d)) on ACT;
            # meanwhile DVE poisons the gram diagonals with -1e38 (the
            # logsumexp diag-kill, done here instead of a post-msub mask
            # add so it rides the ACT window off the critical path; no
            # -10 shift is needed since logits are bounded by 10)
            tln = work.tile([50, NB], f32, tag="tln")
            nc.scalar.activation(tln[:], dsq[:], Act.Ln, scale=0.1)
            inv = work.tile([50, NB], f32, tag="inv")
            nc.scalar.activation(inv[:], tln[:], Act.Exp, scale=-0.5)
            gpM = work.tile([50, 200], f32, tag="gpM")
            nc.vector.scalar_tensor_tensor(
                blk(gpM[:]), identB, -1e38, blk(gp[:]), op0=Alu.mult, op1=Alu.add
            )

            # S = G * inv_row * inv_col; col-broadcast via ones^T @ diag(inv).
            # dinv first: the PE matmul it feeds overlaps the DVE row-scale.
            invrep = inv[:].unsqueeze(2).broadcast_to([50, NB, 50])
            dinv = work.tile([50, 200], bf16, tag="dinv")
            nc.vector.tensor_mul(blk(dinv[:]), identB, invrep)
            binv4 = psb.tile([50, 200], f32, tag="binv4")
            nc.tensor.matmul(binv4[:], cb[:], dinv[:], start=True, stop=True)
            rowsc = work.tile([50, 200], f32, tag="rowsc")
            nc.vector.tensor_mul(blk(rowsc[:]), blk(gpM[:]), invrep)
            msub = work.tile([50, 200], f32, tag="msub")
            nc.vector.tensor_mul(msub[:], rowsc[:], binv4[:])

            eall = work.tile([50, 200], bf16, tag="eall")
            nc.scalar.activation(eall[:], msub[:], Act.Exp)
            esum = work.tile([50, NB], f32, tag="esum")
            nc.vector.reduce_sum(esum[:], blk(eall[:]), axis=X)
            nc.scalar.activation(cmb[:, 0:4], esum[:], Act.Ln)

            # positives (diag of each pair's (0:25, 25:50) block): the DVE
            # runs them during the exp/ln activations; their DMA rides the
            # idle gpsimd queue so its HBM receipt overlaps the lnes DMA
            pmul = work.tile([25, NB * 25], f32, tag="pmul")
            nc.vector.tensor_mul(
                blk(pmul[:], f=25),
                blk(msub[0:25, :])[:, :, 25:50],
                ident[0:25, 0:25].unsqueeze(1).broadcast_to([25, NB, 25]),
            )
            nc.vector.reduce_sum(cmb[0:25, 4:8], blk(pmul[:], f=25), axis=X)
            nc.gpsimd.dma_start(out_dram[:, 4:8], cmb[:, 4:8])

            nc.sync.dma_start(out_dram[:, 0:4], cmb[:, 0:4])

    nc.compile()
    return nc


def get_nc():
    global _nc_cache
    if _nc_cache is None:
        _nc_cache = _build_nc()
    return _nc_cache


def pack_inputs(proj: np.ndarray) -> np.ndarray:
    """(96,256,64,64) -> (128, 32, 8, 50) bf16: partition=c%128,
    free=(pair, chunk k=(cb,dy,dx), view, region rh*5+rw)."""
    win = np.array([[c - 1, c] for c in _CENTRES])  # (5, 2): rows/cols of window
    v = np.stack([proj[32:64], proj[64:96]], axis=1)  # (32, 2, 256, 64, 64)
    g = v[:, :, :, win[:, :, None, None], win[None, None, :, :]]  # (32,2,256,5,2,5,2)
    g = g.reshape(32, 2, 2, 128, 5, 2, 5, 2)  # b, view, cb, c', rh, dy, rw, dx
    arr = np.transpose(g, (3, 0, 2, 5, 7, 1, 4, 6))  # c', b, cb, dy, dx, view, rh, rw
    return np.ascontiguousarray(arr).reshape(128, 32, 8, 50).astype(ml_dtypes.bfloat16)


def kernel(proj: np.ndarray) -> np.ndarray:
    from concourse.bass_utils import run_bass_kernel_spmd

    nc = get_nc()
    arr = pack_inputs(np.asarray(proj))
    cf = np.eye(50, dtype=np.float32)
    in_maps = [
        {
            "u": np.ascontiguousarray(arr[:, c * NB : (c + 1) * NB]).reshape(128, FREE),
            "cf": cf,
        }
        for c in range(NCORES)
    ]
    results = run_bass_kernel_spmd(nc, in_maps, list(range(NCORES))).results
    total = 0.0
    for r in results:
        out = np.asarray(r["out"], dtype=np.float64)  # (50, 8)
        lnes = out[:, 0:4]          # lse - 10 per (row, pair)
        posf = out[0:25, 4:8]       # positive logits - 10 per (region, pair)
        total += lnes.sum() - 2.0 * posf.sum()
    return np.float32(total / (2.0 * R * NB * NCORES))
